# revision 14
# baseline (speedup 1.0000x reference)
"""Trainium2 Bass kernel for a sparse (sliding-window) attention layer.

Reference computation (B=2, S=2048, D=2048, H=16 heads, window=256, fp32):
    qp = q @ Wq + bq ; kp = k @ Wk + bk ; vp = v @ Wv + bv
    per-head scores with mask (0 <= q_idx - k_idx <= 256), softmax, ctx
    out = merge_heads(ctx) @ Wo + bo
    returns (out, kp, vp)

Sharding: 8 cores = 2 (batch) x 4 (head groups of 4 heads / 512 dims).
Each core computes its batch's projections for its 512 output dims
(transposed layout for q/k so attention feeds straight into the PE),
the windowed attention for its 4 heads, and a partial out-projection
(rows of Wo owned by its heads).  Host sums the 4 partial outputs per
batch (the "out_proj all-reduce") and concatenates kp/vp slices.

The kernel is a 4-round pipeline over 512-token seq chunks: each round
projects q/k/v for the chunk, runs the windowed attention for the
chunk's 4 query blocks on all 4 heads, and emits the chunk's partial
out-projection.  This keeps the PE array streaming continuously (no
HAM re-throttle) and spreads DVE/ACT/DMA work evenly.

Engine budget choices (PE is the bottleneck at ~90% busy, so every
non-GEMM op is pushed off the Tensor engine):
  - the sliding-window mask is ADDED ON THE VECTOR ENGINE (one
    tensor_tensor on the score PSUM against a static fp32 band-mask
    tile) instead of PE identity-matmuls
  - the prob transposes (row-major softmax probs -> k-major for the
    ctx matmul) stay on the PE (a DMA-XBAR transpose was tried and
    runs ~1.2us SERIALIZED on the issuing hwdge queue - far worse)
  - the padded prob buffers are zeroed ONCE: pad blocks are never
    written by any round
  - host-side blocked DRAM layouts give every big DMA 4-16KB
    contiguous per-partition lines (fewer descriptors, faster ramp)
  - identity/masks come from DRAM (no gpsimd make_identity on the
    startup critical path); input doorbells ride the sync queue while
    weight/output doorbells ride the scalar queue
  - kp/vp/pout DRAM outputs are bf16 (host upconverts); matmuls are
    bf16 with fp32 PSUM accumulation; softmax statistics stay fp32
"""

import os
import sys

import numpy as np

B = 2
S = 2048
D = 2048
GD = 512          # dims per core (4 heads x 128)
NH = 4            # heads per core
P = 128
WIN = 256         # sliding window
NDB = D // P      # 16 contraction blocks
SC = 512          # seq chunk (one pipeline round)
NSC = S // SC     # 4 rounds
NSB = S // P      # 16 seq blocks
SCALE = 1.0 / np.sqrt(P)

_CACHE = {}
LAST_RESULTS = None


def _mm_dtype_name():
    return os.environ.get("KERNEL_MM_DT", "bf16")


def _build_nc():
    sys.path.insert(0, "/opt/trn_rl_repo")
    import concourse.bass as bass  # noqa: F401
    import concourse.tile as tile
    from concourse import mybir, bacc
    from contextlib import ExitStack

    F32 = mybir.dt.float32
    CDT = mybir.dt.bfloat16 if _mm_dtype_name() == "bf16" else F32

    nc = bacc.Bacc("TRN2", target_bir_lowering=False, debug=False, num_devices=8)

    NQ = 4            # weight/x sub-tiles (4 db blocks each)

    # blocked DRAM layouts: 4KB+ contiguous per-partition lines
    xq_b = nc.dram_tensor("xq_b", [NSC, NQ, P, 4 * SC], CDT, kind="ExternalInput")
    xk_b = nc.dram_tensor("xk_b", [NSC, NQ, P, 4 * SC], CDT, kind="ExternalInput")
    xv_b = nc.dram_tensor("xv_b", [NSC, NQ, P, 4 * SC], CDT, kind="ExternalInput")
    wq_b = nc.dram_tensor("wq_b", [NQ, P, 4 * GD], CDT, kind="ExternalInput")
    wk_b = nc.dram_tensor("wk_b", [NQ, P, 4 * GD], CDT, kind="ExternalInput")
    wv_b = nc.dram_tensor("wv_b", [P, NDB * GD], CDT, kind="ExternalInput")
    wo_b = nc.dram_tensor("wo_b", [P, NH * D], CDT, kind="ExternalInput")
    bq2 = nc.dram_tensor("bq2", [P, NH], F32, kind="ExternalInput")
    bk2 = nc.dram_tensor("bk2", [P, NH], F32, kind="ExternalInput")
    bvb = nc.dram_tensor("bvb", [P, GD], F32, kind="ExternalInput")
    maskd = nc.dram_tensor("maskd", [P, 3 * P], F32, kind="ExternalInput")
    identd = nc.dram_tensor("identd", [P, P], CDT, kind="ExternalInput")

    kpT_o = nc.dram_tensor("kpT", [GD, S], CDT, kind="ExternalOutput")
    vp_o = nc.dram_tensor("vp", [S, GD], CDT, kind="ExternalOutput")
    pout_o = nc.dram_tensor("pout", [NSB, P, D], CDT, kind="ExternalOutput")

    kpT_r = kpT_o.ap().rearrange("(h p) s -> h p s", p=P)

    AluOp = mybir.AluOpType
    ActFn = mybir.ActivationFunctionType

    with tile.TileContext(nc) as tc, ExitStack() as top:
        const = top.enter_context(tc.tile_pool(name="const", bufs=1))
        ident = const.tile([P, P], CDT, name="ident")
        nc.scalar.dma_start(ident[:], identd.ap())
        # remaining const doorbells are deferred until after the first
        # weight quarters so the scalar hwdge queue services the
        # startup-critical transfers first
        mask_sb = const.tile([P, 3 * P], F32, name="mask_sb")
        bq_sb = const.tile([P, NH], F32, name="bq_sb")
        bk_sb = const.tile([P, NH], F32, name="bk_sb")
        bvb_sb = const.tile([P, GD], F32, name="bvb_sb")

        # weights: q/k split into 4 sub-tiles so the first matmuls can
        # start as soon as the first quarter + first x quarter land.
        # Weight doorbells go on the scalar queue, x doorbells on the
        # sync queue, so both streams ramp in parallel.
        wpool = top.enter_context(tc.tile_pool(name="wpool", bufs=1))
        wq_sb = [wpool.tile([P, NDB // NQ, GD], CDT, name=f"wq_sb{i}")
                 for i in range(NQ)]
        wk_sb = [wpool.tile([P, NDB // NQ, GD], CDT, name=f"wk_sb{i}")
                 for i in range(NQ)]
        wv_sb = wpool.tile([P, NDB, GD], CDT, name="wv_sb")
        wo_sb = wpool.tile([P, NH, D], CDT, name="wo_sb")

        # long-lived activations
        persist1 = top.enter_context(tc.tile_pool(name="persist1", bufs=1))
        qpT = [persist1.tile([P, S], CDT, name=f"qpT{h}") for h in range(NH)]
        kpT = [persist1.tile([P, S], CDT, name=f"kpT{h}") for h in range(NH)]
        persist2 = top.enter_context(tc.tile_pool(name="persist2", bufs=1))
        vpB = [persist2.tile([P, GD], CDT, name=f"vpB{sb}") for sb in range(NSB)]
        persist3 = top.enter_context(tc.tile_pool(name="persist3", bufs=1))
        ctxT = [persist3.tile([P, S], CDT, name=f"ctxT{h}") for h in range(NH)]

        # transposed-prob buffers: [key-block rel 0..5, query 0..511].
        # Pad blocks (rel<t or rel>t+2) are never written by any round,
        # so a single memset keeps them zero for the whole kernel.
        persist4 = top.enter_context(tc.tile_pool(name="persist4", bufs=1))
        pbufs = [persist4.tile([P, 6, SC], CDT, name=f"pbuf{i}") for i in range(2)]
        for pb in pbufs:
            nc.gpsimd.memset(pb[:], 0.0)

        # working pools
        xpool = top.enter_context(tc.tile_pool(name="xpool", bufs=2))
        wkp = top.enter_context(tc.tile_pool(name="wkp", bufs=8))
        cpool = top.enter_context(tc.tile_pool(name="cpool", bufs=6))
        # PSUM: 2 (proj) + 3 (scores) + 2 (transpose) + 1 (ctx) = 8 banks
        psP = top.enter_context(tc.tile_pool(name="psP", bufs=2, space="PSUM"))
        psb = top.enter_context(tc.tile_pool(name="psb", bufs=3, space="PSUM"))
        pst = top.enter_context(tc.tile_pool(name="pst", bufs=2, space="PSUM"))
        psc = top.enter_context(tc.tile_pool(name="psc", bufs=1, space="PSUM"))

        # warm the PE (HAM un-throttles after ~3.4us of activity) with
        # garbage matmuls on the identity tile while the real input
        # DMAs are still in flight
        warm = psP.tile([P, SC], F32, tag="proj", name="warm")
        for _ in range(16):
            nc.tensor.matmul(warm[:, :P], lhsT=ident[:],
                             rhs=ident[:], start=True, stop=True)

        def emit_out_proj_sb(sb):
            # partial out-projection for one seq block (emitted one
            # round late, interleaved per attention head: its matmuls
            # are always-ready PE work that absorbs the attention
            # phase's softmax latency in the in-order PE queue)
            for ec in range(D // SC):
                psq = psP.tile([P, SC], F32, tag="proj", name="psq")
                for h in range(NH):
                    nc.tensor.matmul(
                        psq[:],
                        lhsT=ctxT[h][:, sb * P:(sb + 1) * P],
                        rhs=wo_sb[:, h, ec * SC:(ec + 1) * SC],
                        start=(h == 0),
                        stop=(h == NH - 1),
                    )
                po = cpool.tile([P, SC], CDT, tag="po", name="po")
                if ec % 2 == 0:
                    nc.scalar.copy(po[:], psq[:])
                else:
                    nc.vector.tensor_copy(po[:], psq[:])
                nc.scalar.dma_start(
                    pout_o.ap()[sb, :, ec * SC:(ec + 1) * SC], po[:])

        def load_x_chunk(x_b, sc):
            subs = []
            for i in range(NQ):
                t = xpool.tile([P, NDB // NQ, SC], CDT, tag=f"x{i}", name=f"x{i}")
                nc.sync.dma_start(t[:], x_b.ap()[sc, i])
                subs.append(t)
            return subs

        xs_next = None
        for sc in range(NSC):
            ssl = slice(sc * SC, (sc + 1) * SC)
            kpT_writes = []

            # ---- q / k projections for this chunk (transposed layout) ----
            for x_b, w_b, w_sb, b_sb, dstT, is_k in (
                (xq_b, wq_b, wq_sb, bq_sb, qpT, False),
                (xk_b, wk_b, wk_sb, bk_sb, kpT, True),
            ):
                if sc == 0:
                    # interleave weight-quarter / x-quarter transfers
                    xs = []
                    for i in range(NQ):
                        nc.scalar.dma_start(w_sb[i][:], w_b.ap()[i])
                        t = xpool.tile([P, NDB // NQ, SC], CDT,
                                       tag=f"x{i}", name=f"x{i}")
                        nc.sync.dma_start(t[:], x_b.ap()[0, i])
                        xs.append(t)
                    if not is_k:
                        # bias doorbells MUST be emitted before their
                        # first consumer (deps follow program order)
                        nc.scalar.dma_start(bq_sb[:], bq2.ap())
                        nc.scalar.dma_start(bk_sb[:], bk2.ap())
                else:
                    xs = xs_next[0 if not is_k else 1]
                for hb in range(NH):
                    ps = psP.tile([P, SC], F32, tag="proj", name="ps")
                    for db in range(NDB):
                        nc.tensor.matmul(
                            ps[:],
                            lhsT=w_sb[db // 4][:, db % 4, hb * P:(hb + 1) * P],
                            rhs=xs[db // 4][:, db % 4, :],
                            start=(db == 0),
                            stop=(db == NDB - 1),
                        )
                    nc.vector.tensor_scalar_add(
                        dstT[hb][:, ssl], ps[:], b_sb[:, hb:hb + 1])
                    if is_k:
                        # doorbell deferred: a dependent output DMA at
                        # the head of the scalar queue would block the
                        # wv/wo/const doorbells behind it
                        kpT_writes.append(hb)

            # ---- v projection for this chunk (natural layout) ----
            if sc == 0:
                nc.scalar.dma_start(wv_sb[:], wv_b.ap())
                nc.scalar.dma_start(wo_sb[:], wo_b.ap())
                nc.scalar.dma_start(bvb_sb[:], bvb.ap())
                nc.scalar.dma_start(mask_sb[:], maskd.ap())
                xvs = load_x_chunk(xv_b, 0)
            else:
                xvs = xs_next[2]
            for hb in kpT_writes:
                nc.scalar.dma_start(kpT_r[hb][:, ssl], kpT[hb][:, ssl])
            for s2 in range(SC // P):
                sb = sc * (SC // P) + s2
                ps2 = psP.tile([P, GD], F32, tag="proj", name="ps2")
                for db in range(NDB):
                    nc.tensor.matmul(
                        ps2[:],
                        lhsT=xvs[db // 4][:, db % 4, s2 * P:(s2 + 1) * P],
                        rhs=wv_sb[:, db, :],
                        start=(db == 0),
                        stop=(db == NDB - 1),
                    )
                nc.vector.tensor_tensor(vpB[sb][:], ps2[:], bvb_sb[:], AluOp.add)
                nc.scalar.dma_start(vp_o.ap()[sb * P:(sb + 1) * P, :], vpB[sb][:])

            # prefetch next chunk's x while attention runs
            if sc + 1 < NSC:
                xs_next = (load_x_chunk(xq_b, sc + 1),
                           load_x_chunk(xk_b, sc + 1),
                           load_x_chunk(xv_b, sc + 1))

            # ---- windowed attention for this chunk's 4 query blocks ----
            qc = sc
            kjbase = 4 * qc - 2

            def emit_transp_ctx(h, pb, probBs):
                # transposes grouped per destination key-block, then
                # the ctx matmuls; called one head LATE so head h+1's
                # always-ready score matmuls sit ahead of these
                # softmax-gated instructions in the PE queue
                for rel in range(2 if qc == 0 else 0, 6):
                    t0 = max(0, rel - 2)
                    t1 = min(3, rel)
                    ps_t = pst.tile([P, 3 * P], CDT, tag="ps_t", name="ps_t")
                    for t in range(t0, t1 + 1):
                        probB, kj_lo = probBs[t]
                        j = (kjbase + rel) - kj_lo // P
                        nc.tensor.transpose(
                            ps_t[:, (t - t0) * P:(t - t0 + 1) * P],
                            probB[:, j * P:(j + 1) * P],
                            ident[:])
                    nc.vector.tensor_copy(
                        pb[:, rel, t0 * P:(t1 + 1) * P],
                        ps_t[:, 0:(t1 - t0 + 1) * P])
                ps_pv = psc.tile([P, SC], F32, tag="ps_pv", name="ps_pv")
                for pr in range(2):
                    cs = slice(pr * 2 * P, (pr + 1) * 2 * P)
                    rels = [r for r in range(2 * pr, 2 * pr + 4)
                            if kjbase + r >= 0]
                    for i2, r in enumerate(rels):
                        nc.tensor.matmul(
                            ps_pv[:, cs],
                            lhsT=vpB[kjbase + r][:, h * P:(h + 1) * P],
                            rhs=pb[:, r, cs],
                            start=(i2 == 0),
                            stop=(i2 == len(rels) - 1),
                        )
                nc.scalar.copy(ctxT[h][:, qc * SC:(qc + 1) * SC], ps_pv[:])

            prev = None
            for h in range(NH):
                pb = pbufs[(qc * NH + h) % 2]
                probBs = []
                for t in range(4):
                    qb = qc * 4 + t
                    qs = qb * P
                    kj_lo = max(0, qs - WIN)
                    wdt = qs + P - kj_lo          # 128 / 256 / 384
                    ps_s = psb.tile([P, 3 * P], F32, tag="ps_s", name="ps_s")
                    nc.tensor.matmul(
                        ps_s[:, :wdt],
                        lhsT=qpT[h][:, qs:qs + P],
                        rhs=kpT[h][:, kj_lo:kj_lo + wdt],
                        start=True,
                        stop=True,
                    )
                    # sliding-window mask on the Vector engine (the
                    # right-aligned slice of the static band mask
                    # matches every wdt)
                    nc.vector.tensor_tensor(
                        ps_s[:, :wdt], ps_s[:, :wdt],
                        mask_sb[:, 3 * P - wdt:], AluOp.add)
                    exps = wkp.tile([P, 3 * P], CDT, tag="exps", name="exps")
                    rsum = wkp.tile([P, 1], F32, tag="rsum", name="rsum")
                    nc.scalar.activation(exps[:, :wdt], ps_s[:, :wdt],
                                         ActFn.Exp, scale=float(SCALE),
                                         accum_out=rsum[:])
                    rinv = wkp.tile([P, 1], F32, tag="rinv", name="rinv")
                    nc.vector.reciprocal(rinv[:], rsum[:])
                    probB = wkp.tile([P, 3 * P], CDT, tag="probB", name="probB")
                    nc.vector.tensor_scalar_mul(probB[:, :wdt],
                                                exps[:, :wdt], rinv[:])
                    probBs.append((probB, kj_lo))
                if sc > 0:
                    emit_out_proj_sb((sc - 1) * (SC // P) + h)
                if prev is not None:
                    emit_transp_ctx(*prev)
                prev = (h, pb, probBs)
            emit_transp_ctx(*prev)

        for s2 in range(SC // P):
            emit_out_proj_sb((NSC - 1) * (SC // P) + s2)

    nc.compile()
    return nc


def _band_mask():
    i = np.arange(P)[:, None]
    j = np.arange(3 * P)[None, :]
    return np.where((j >= i) & (j <= i + WIN), 0.0, -1e6).astype(np.float32)


def kernel(q, k, v, Wq, bq, Wk, bk, Wv, bv, Wo, bo):
    global LAST_RESULTS
    q = np.asarray(q, np.float32)
    k = np.asarray(k, np.float32)
    v = np.asarray(v, np.float32)
    Wq = np.asarray(Wq, np.float32)
    Wk = np.asarray(Wk, np.float32)
    Wv = np.asarray(Wv, np.float32)
    Wo = np.asarray(Wo, np.float32)
    bq = np.asarray(bq, np.float32)
    bk = np.asarray(bk, np.float32)
    bv = np.asarray(bv, np.float32)
    bo = np.asarray(bo, np.float32)

    if "nc" not in _CACHE:
        _CACHE["nc"] = _build_nc()
    nc = _CACHE["nc"]
    from concourse.bass_utils import run_bass_kernel_spmd

    if _mm_dtype_name() == "bf16":
        import ml_dtypes

        cdt = ml_dtypes.bfloat16
    else:
        cdt = np.float32

    NQ = 4
    mask = _band_mask()
    ident = np.eye(P, dtype=cdt)

    def x_block(x):
        # x[b] (S, D) -> blocked [NSC, NQ, P, 4*SC]: per-partition
        # lines are 4KB contiguous (one descriptor per partition)
        xT = x.T.astype(cdt)  # (D, S)
        return np.ascontiguousarray(
            xT.reshape(NQ, 4, P, NSC, SC).transpose(3, 0, 2, 1, 4)
            .reshape(NSC, NQ, P, 4 * SC))

    def w_block(w):
        # W slice (D, GD) -> [NQ, P, 4*GD]
        return np.ascontiguousarray(
            w.astype(cdt).reshape(NQ, 4, P, GD).transpose(0, 2, 1, 3)
            .reshape(NQ, P, 4 * GD))

    xB = {}
    for b in range(B):
        xB[("q", b)] = x_block(q[b])
        xB[("k", b)] = x_block(k[b])
        xB[("v", b)] = x_block(v[b])

    in_maps = []
    for core in range(8):
        b, g = divmod(core, 4)
        sl = slice(g * GD, (g + 1) * GD)
        in_maps.append({
            "xq_b": xB[("q", b)],
            "xk_b": xB[("k", b)],
            "xv_b": xB[("v", b)],
            "wq_b": w_block(Wq[:, sl]),
            "wk_b": w_block(Wk[:, sl]),
            "wv_b": np.ascontiguousarray(
                Wv[:, sl].astype(cdt).reshape(NDB, P, GD).transpose(1, 0, 2)
                .reshape(P, NDB * GD)),
            "wo_b": np.ascontiguousarray(
                Wo[sl, :].astype(cdt).reshape(NH, P, D).transpose(1, 0, 2)
                .reshape(P, NH * D)),
            "bq2": np.ascontiguousarray(bq[sl].reshape(NH, P).T),
            "bk2": np.ascontiguousarray(bk[sl].reshape(NH, P).T),
            "bvb": np.ascontiguousarray(np.broadcast_to(bv[sl], (P, GD))),
            "maskd": mask,
            "identd": ident,
        })

    trace = os.environ.get("KERNEL_TRACE", "0") == "1"
    res = run_bass_kernel_spmd(nc, in_maps, core_ids=list(range(8)), trace=trace)
    LAST_RESULTS = res

    out = np.zeros((B, S, D), np.float64)
    kp = np.empty((B, S, D), np.float32)
    vp = np.empty((B, S, D), np.float32)
    for core in range(8):
        b, g = divmod(core, 4)
        sl = slice(g * GD, (g + 1) * GD)
        r = res.results[core]
        kp[b][:, sl] = r["kpT"].astype(np.float32).T
        vp[b][:, sl] = r["vp"].astype(np.float32)
        out[b] += r["pout"].reshape(S, D).astype(np.float64)
    out = (out + bo.astype(np.float64)).astype(np.float32)
    return out, kp, vp


# revision 39
# speedup vs baseline: 1.1873x; 1.1873x over previous
"""Trainium2 Bass kernel for a sparse (sliding-window) attention layer.

Reference computation (B=2, S=2048, D=2048, H=16 heads, window=256, fp32):
    qp = q @ Wq + bq ; kp = k @ Wk + bk ; vp = v @ Wv + bv
    per-head scores with mask (0 <= q_idx - k_idx <= 256), softmax, ctx
    out = merge_heads(ctx) @ Wo + bo
    returns (out, kp, vp)

Sharding: 8 cores = 2 (batch) x 4 (head groups of 4 heads / 512 dims).
Each core computes its batch's projections for its 512 output dims
(transposed layout for q/k so attention feeds straight into the PE),
the windowed attention for its 4 heads, and a partial out-projection
(rows of Wo owned by its heads).  Host sums the 4 partial outputs per
batch (the "out_proj all-reduce") and concatenates kp/vp slices.

The kernel is a 4-round pipeline over 512-token seq chunks: each round
projects q/k/v for the chunk, runs the windowed attention for the
chunk's 4 query blocks on all 4 heads, and emits the chunk's partial
out-projection.  This keeps the PE array streaming continuously (no
HAM re-throttle) and spreads DVE/ACT/DMA work evenly.

Engine budget choices (PE is the bottleneck at ~90% busy, so every
non-GEMM op is pushed off the Tensor engine):
  - the sliding-window mask is ADDED ON THE VECTOR ENGINE (one
    tensor_tensor on the score PSUM against a static fp32 band-mask
    tile) instead of PE identity-matmuls
  - the prob transposes (row-major softmax probs -> k-major for the
    ctx matmul) stay on the PE (a DMA-XBAR transpose was tried and
    runs ~1.2us SERIALIZED on the issuing hwdge queue - far worse)
  - the padded prob buffers are zeroed ONCE: pad blocks are never
    written by any round
  - host-side blocked DRAM layouts give every big DMA 4-16KB
    contiguous per-partition lines (fewer descriptors, faster ramp)
  - identity/masks come from DRAM (no gpsimd make_identity on the
    startup critical path); input doorbells ride the sync queue while
    weight/output doorbells ride the scalar queue
  - kp/vp/pout DRAM outputs are bf16 (host upconverts); matmuls are
    bf16 with fp32 PSUM accumulation; softmax statistics stay fp32
"""

import os
import sys

import numpy as np

B = 2
S = 2048
D = 2048
GD = 512          # dims per core (4 heads x 128)
NH = 4            # heads per core
P = 128
WIN = 256         # sliding window
NDB = D // P      # 16 contraction blocks
SC = 512          # seq chunk (one pipeline round)
NSC = S // SC     # 4 rounds
NSB = S // P      # 16 seq blocks
SCALE = 1.0 / np.sqrt(P)

_CACHE = {}
LAST_RESULTS = None


def _mm_dtype_name():
    return os.environ.get("KERNEL_MM_DT", "bf16")


def _build_nc():
    sys.path.insert(0, "/opt/trn_rl_repo")
    import concourse.bass as bass  # noqa: F401
    import concourse.tile as tile
    from concourse import mybir, bacc
    from contextlib import ExitStack

    F32 = mybir.dt.float32
    CDT = mybir.dt.bfloat16 if _mm_dtype_name() == "bf16" else F32

    nc = bacc.Bacc("TRN2", target_bir_lowering=False, debug=False, num_devices=8)

    NQ = 4            # weight/x sub-tiles (4 db blocks each)

    # blocked DRAM layouts: 4KB+ contiguous per-partition lines
    xq_b = nc.dram_tensor("xq_b", [NSC, NQ, P, 4 * SC], CDT, kind="ExternalInput")
    xk_b = nc.dram_tensor("xk_b", [NSC, NQ, P, 4 * SC], CDT, kind="ExternalInput")
    xv_b = nc.dram_tensor("xv_b", [NSC, NQ, P, 4 * SC], CDT, kind="ExternalInput")
    wq_b = nc.dram_tensor("wq_b", [NQ, P, 4 * GD], CDT, kind="ExternalInput")
    wk_b = nc.dram_tensor("wk_b", [NQ, P, 4 * GD], CDT, kind="ExternalInput")
    wv_b = nc.dram_tensor("wv_b", [P, NDB * GD], CDT, kind="ExternalInput")
    wo_b = nc.dram_tensor("wo_b", [P, NH * D], CDT, kind="ExternalInput")
    bq2 = nc.dram_tensor("bq2", [P, NH], F32, kind="ExternalInput")
    bk2 = nc.dram_tensor("bk2", [P, NH], F32, kind="ExternalInput")
    bvb = nc.dram_tensor("bvb", [P, GD], F32, kind="ExternalInput")
    maskd = nc.dram_tensor("maskd", [P, 3 * P], F32, kind="ExternalInput")
    identd = nc.dram_tensor("identd", [P, P], CDT, kind="ExternalInput")

    kpT_o = nc.dram_tensor("kpT", [GD, S], CDT, kind="ExternalOutput")
    vp_o = nc.dram_tensor("vp", [S, GD], CDT, kind="ExternalOutput")
    pout_o = nc.dram_tensor("pout", [NSB, P, D], CDT, kind="ExternalOutput")

    kpT_r = kpT_o.ap().rearrange("(h p) s -> h p s", p=P)

    AluOp = mybir.AluOpType
    ActFn = mybir.ActivationFunctionType

    with tile.TileContext(nc) as tc, ExitStack() as top:
        const = top.enter_context(tc.tile_pool(name="const", bufs=1))
        ident = const.tile([P, P], CDT, name="ident")
        junk = const.tile([P, P], CDT, name="junk")
        nc.gpsimd.memset(junk[:], 0.125)
        nc.scalar.dma_start(ident[:], identd.ap())
        # remaining const doorbells are deferred until after the first
        # weight quarters so the scalar hwdge queue services the
        # startup-critical transfers first
        mask_sb = const.tile([P, 3 * P], F32, name="mask_sb")
        bq_sb = const.tile([P, NH], F32, name="bq_sb")
        bk_sb = const.tile([P, NH], F32, name="bk_sb")
        bvb_sb = const.tile([P, GD], F32, name="bvb_sb")

        # weights: q/k split into 4 sub-tiles so the first matmuls can
        # start as soon as the first quarter + first x quarter land.
        # Weight doorbells go on the scalar queue, x doorbells on the
        # sync queue, so both streams ramp in parallel.
        wpool = top.enter_context(tc.tile_pool(name="wpool", bufs=1))
        wq_sb = [wpool.tile([P, NDB // NQ, GD], CDT, name=f"wq_sb{i}")
                 for i in range(NQ)]
        wk_sb = [wpool.tile([P, NDB // NQ, GD], CDT, name=f"wk_sb{i}")
                 for i in range(NQ)]
        wv_sb = wpool.tile([P, NDB, GD], CDT, name="wv_sb")
        wo_sb = wpool.tile([P, NH, D], CDT, name="wo_sb")

        # long-lived activations
        persist1 = top.enter_context(tc.tile_pool(name="persist1", bufs=1))
        qpT = [persist1.tile([P, S], CDT, name=f"qpT{h}") for h in range(NH)]
        kpT = [persist1.tile([P, S], CDT, name=f"kpT{h}") for h in range(NH)]
        persist2 = top.enter_context(tc.tile_pool(name="persist2", bufs=1))
        vpB = [persist2.tile([P, GD], CDT, name=f"vpB{sb}") for sb in range(NSB)]
        persist3 = top.enter_context(tc.tile_pool(name="persist3", bufs=1))
        ctxT = [persist3.tile([P, S], CDT, name=f"ctxT{h}") for h in range(NH)]

        # transposed-prob buffers: [key-block rel 0..5, query 0..511].
        # Pad blocks (rel<t or rel>t+2) are never written by any round,
        # so a single memset keeps them zero for the whole kernel.
        persist4 = top.enter_context(tc.tile_pool(name="persist4", bufs=1))
        pbufs = [persist4.tile([P, 6, SC], CDT, name=f"pbuf{i}") for i in range(2)]
        for pb in pbufs:
            nc.gpsimd.memset(pb[:], 0.0)

        # working pools
        xpool = top.enter_context(tc.tile_pool(name="xpool", bufs=2))
        wkp = top.enter_context(tc.tile_pool(name="wkp", bufs=8))
        cpool = top.enter_context(tc.tile_pool(name="cpool", bufs=6))
        # PSUM: 4 (proj/out-proj) + 2 (scores) + 2 (transpose+ctx) = 8
        # banks.  4 proj banks let chunk 0 run four concurrent hb
        # accumulation chains paced by the arriving weight/x quarters,
        # and give the DVE bias-add/copy consumers 3 chains of slack.
        psP = top.enter_context(tc.tile_pool(name="psP", bufs=4, space="PSUM"))
        psb = top.enter_context(tc.tile_pool(name="psb", bufs=2, space="PSUM"))
        pst = top.enter_context(tc.tile_pool(name="pst", bufs=2, space="PSUM"))

        # warm the PE (HAM un-throttles after ~3.4us of activity) with
        # garbage matmuls on a memset tile: gpsimd can produce it at
        # ~6us (engine program start) while the ident DMA only lands at
        # ~10us, so warm-up begins ~4us earlier
        warm = psP.tile([P, SC], F32, tag="proj", name="warm")
        for _ in range(28):
            nc.tensor.matmul(warm[:, :P], lhsT=junk[:],
                             rhs=junk[:], start=True, stop=True)

        def emit_out_proj_sb(sb):
            # partial out-projection for one seq block (emitted one
            # round late, interleaved per attention head: its matmuls
            # are always-ready PE work that absorbs the attention
            # phase's softmax latency in the in-order PE queue)
            for ec in range(D // SC):
                psq = psP.tile([P, SC], F32, tag="proj", name="psq")
                for h in range(NH):
                    nc.tensor.matmul(
                        psq[:],
                        lhsT=ctxT[h][:, sb * P:(sb + 1) * P],
                        rhs=wo_sb[:, h, ec * SC:(ec + 1) * SC],
                        start=(h == 0),
                        stop=(h == NH - 1),
                    )
                po = cpool.tile([P, SC], CDT, tag="po", name="po")
                if ec % 2 == 0:
                    nc.scalar.copy(po[:], psq[:])
                else:
                    nc.vector.tensor_copy(po[:], psq[:])
                nc.scalar.dma_start(
                    pout_o.ap()[sb, :, ec * SC:(ec + 1) * SC], po[:])

        def load_x_chunk(x_b, sc):
            subs = []
            for i in range(NQ):
                t = xpool.tile([P, NDB // NQ, SC], CDT, tag=f"x{i}", name=f"x{i}")
                nc.sync.dma_start(t[:], x_b.ap()[sc, i])
                subs.append(t)
            return subs

        def qproj_chain(sc, xs, hb):
            # one q-projection hb chain for chunk sc (interleaved into
            # the PREVIOUS chunk's attention as always-ready PE filler)
            ps = psP.tile([P, SC], F32, tag="proj", name="ps")
            for db in range(NDB):
                nc.tensor.matmul(
                    ps[:],
                    lhsT=wq_sb[db // 4][:, db % 4, hb * P:(hb + 1) * P],
                    rhs=xs[db // 4][:, db % 4, :],
                    start=(db == 0),
                    stop=(db == NDB - 1),
                )
            nc.scalar.activation(qpT[hb][:, sc * SC:(sc + 1) * SC], ps[:],
                                 ActFn.Identity, bias=bq_sb[:, hb:hb + 1])

        xs_next = None
        for sc in range(NSC):
            ssl = slice(sc * SC, (sc + 1) * SC)
            kpT_writes = []

            # ---- q / k projections for this chunk (transposed layout) ----
            # (for sc>0 the q projection was already emitted, interleaved
            # into chunk sc-1's attention)
            for x_b, w_b, w_sb, b_sb, dstT, is_k in (
                (xq_b, wq_b, wq_sb, bq_sb, qpT, False),
                (xk_b, wk_b, wk_sb, bk_sb, kpT, True),
            ):
                if sc > 0 and not is_k:
                    continue
                if sc == 0:
                    # interleave weight-quarter / x-quarter transfers;
                    # the very first quarter goes db-block by db-block
                    # so the first matmul's dependencies are minimal
                    xs = []
                    for i in range(NQ):
                        t = xpool.tile([P, NDB // NQ, SC], CDT,
                                       tag=f"x{i}", name=f"x{i}")
                        if i == 0 and not is_k:
                            for d2 in range(4):
                                csl = slice(d2 * SC, (d2 + 1) * SC)
                                nc.scalar.dma_start(
                                    w_sb[0][:, d2], w_b.ap()[0][:, csl])
                                nc.sync.dma_start(
                                    t[:, d2], x_b.ap()[0, 0][:, csl])
                        else:
                            nc.scalar.dma_start(w_sb[i][:], w_b.ap()[i])
                            nc.sync.dma_start(t[:], x_b.ap()[0, i])
                        xs.append(t)
                    if not is_k:
                        # bias doorbells MUST be emitted before their
                        # first consumer (deps follow program order)
                        nc.scalar.dma_start(bq_sb[:], bq2.ap())
                        nc.scalar.dma_start(bk_sb[:], bk2.ap())
                    # quarter-major: four concurrent hb accumulation
                    # chains, paced by the arriving quarters (a single
                    # hb chain would consume input 4x faster than the
                    # DMA can deliver it)
                    ps_l = [psP.tile([P, SC], F32, tag="proj",
                                     name=f"ps{hb}") for hb in range(NH)]
                    for i in range(NQ):
                        for hb in range(NH):
                            for d2 in range(4):
                                db = 4 * i + d2
                                nc.tensor.matmul(
                                    ps_l[hb][:],
                                    lhsT=w_sb[i][:, d2, hb * P:(hb + 1) * P],
                                    rhs=xs[i][:, d2, :],
                                    start=(db == 0),
                                    stop=(db == NDB - 1),
                                )
                    for hb in range(NH):
                        nc.scalar.activation(dstT[hb][:, ssl], ps_l[hb][:],
                                             ActFn.Identity,
                                             bias=b_sb[:, hb:hb + 1])
                        if is_k:
                            kpT_writes.append(hb)
                    continue
                xs = xs_next[0 if not is_k else 1]
                for hb in range(NH):
                    ps = psP.tile([P, SC], F32, tag="proj", name="ps")
                    for db in range(NDB):
                        nc.tensor.matmul(
                            ps[:],
                            lhsT=w_sb[db // 4][:, db % 4, hb * P:(hb + 1) * P],
                            rhs=xs[db // 4][:, db % 4, :],
                            start=(db == 0),
                            stop=(db == NDB - 1),
                        )
                    nc.scalar.activation(dstT[hb][:, ssl], ps[:],
                                         ActFn.Identity,
                                         bias=b_sb[:, hb:hb + 1])
                    if is_k:
                        # doorbell deferred: a dependent output DMA at
                        # the head of the scalar queue would block the
                        # wv/wo/const doorbells behind it
                        kpT_writes.append(hb)

            # ---- v projection for this chunk (natural layout) ----
            if sc == 0:
                nc.scalar.dma_start(wv_sb[:], wv_b.ap())
                nc.scalar.dma_start(wo_sb[:], wo_b.ap())
                nc.scalar.dma_start(bvb_sb[:], bvb.ap())
                nc.scalar.dma_start(mask_sb[:], maskd.ap())
                xvs = load_x_chunk(xv_b, 0)
            else:
                xvs = xs_next[2]
            for hb in kpT_writes:
                nc.scalar.dma_start(kpT_r[hb][:, ssl], kpT[hb][:, ssl])
            if sc == 0:
                # quarter-major (DMA-paced) like the chunk-0 q/k proj
                ps2_l = [psP.tile([P, GD], F32, tag="proj",
                                  name=f"ps2_{s2}") for s2 in range(SC // P)]
                for i in range(NQ):
                    for s2 in range(SC // P):
                        for d2 in range(4):
                            db = 4 * i + d2
                            nc.tensor.matmul(
                                ps2_l[s2][:],
                                lhsT=xvs[i][:, d2, s2 * P:(s2 + 1) * P],
                                rhs=wv_sb[:, db, :],
                                start=(db == 0),
                                stop=(db == NDB - 1),
                            )
                for s2 in range(SC // P):
                    nc.vector.tensor_tensor(
                        vpB[s2][:], ps2_l[s2][:], bvb_sb[:], AluOp.add)
                    nc.scalar.dma_start(
                        vp_o.ap()[s2 * P:(s2 + 1) * P, :], vpB[s2][:])
            else:
                for s2 in range(SC // P):
                    sb = sc * (SC // P) + s2
                    ps2 = psP.tile([P, GD], F32, tag="proj", name="ps2")
                    for db in range(NDB):
                        nc.tensor.matmul(
                            ps2[:],
                            lhsT=xvs[db // 4][:, db % 4, s2 * P:(s2 + 1) * P],
                            rhs=wv_sb[:, db, :],
                            start=(db == 0),
                            stop=(db == NDB - 1),
                        )
                    nc.vector.tensor_tensor(
                        vpB[sb][:], ps2[:], bvb_sb[:], AluOp.add)
                    nc.scalar.dma_start(
                        vp_o.ap()[sb * P:(sb + 1) * P, :], vpB[sb][:])

            # prefetch next chunk's x while attention runs
            if sc + 1 < NSC:
                xs_next = (load_x_chunk(xq_b, sc + 1),
                           load_x_chunk(xk_b, sc + 1),
                           load_x_chunk(xv_b, sc + 1))

            # ---- windowed attention for this chunk's 4 query blocks ----
            qc = sc
            kjbase = 4 * qc - 2

            def emit_transp_ctx(h, pb, probBs):
                # transposes grouped per destination key-block, then
                # the ctx matmuls; called one head LATE so head h+1's
                # always-ready score matmuls sit ahead of these
                # softmax-gated instructions in the PE queue
                for rel in range(2 if qc == 0 else 0, 6):
                    t0 = max(0, rel - 2)
                    t1 = min(3, rel)
                    ps_t = pst.tile([P, 3 * P], CDT, tag="ps_t", name="ps_t")
                    for t in range(t0, t1 + 1):
                        probB, kj_lo = probBs[t]
                        j = (kjbase + rel) - kj_lo // P
                        nc.tensor.transpose(
                            ps_t[:, (t - t0) * P:(t - t0 + 1) * P],
                            probB[:, j * P:(j + 1) * P],
                            ident[:])
                    nc.vector.tensor_copy(
                        pb[:, rel, t0 * P:(t1 + 1) * P],
                        ps_t[:, 0:(t1 - t0 + 1) * P])
                ps_pv = psP.tile([P, SC], F32, tag="proj", name="ps_pv")
                for pr in range(2):
                    cs = slice(pr * 2 * P, (pr + 1) * 2 * P)
                    rels = [r for r in range(2 * pr, 2 * pr + 4)
                            if kjbase + r >= 0]
                    for i2, r in enumerate(rels):
                        nc.tensor.matmul(
                            ps_pv[:, cs],
                            lhsT=vpB[kjbase + r][:, h * P:(h + 1) * P],
                            rhs=pb[:, r, cs],
                            start=(i2 == 0),
                            stop=(i2 == len(rels) - 1),
                        )
                nc.scalar.copy(ctxT[h][:, qc * SC:(qc + 1) * SC], ps_pv[:])

            prev = None
            for h in range(NH):
                pb = pbufs[(qc * NH + h) % 2]
                probBs = []
                for t in range(4):
                    qb = qc * 4 + t
                    qs = qb * P
                    kj_lo = max(0, qs - WIN)
                    wdt = qs + P - kj_lo          # 128 / 256 / 384
                    ps_s = psb.tile([P, 3 * P], F32, tag="ps_s", name="ps_s")
                    nc.tensor.matmul(
                        ps_s[:, :wdt],
                        lhsT=qpT[h][:, qs:qs + P],
                        rhs=kpT[h][:, kj_lo:kj_lo + wdt],
                        start=True,
                        stop=True,
                    )
                    # sliding-window mask on the Vector engine (the
                    # right-aligned slice of the static band mask
                    # matches every wdt)
                    nc.vector.tensor_tensor(
                        ps_s[:, :wdt], ps_s[:, :wdt],
                        mask_sb[:, 3 * P - wdt:], AluOp.add)
                    exps = wkp.tile([P, 3 * P], CDT, tag="exps", name="exps")
                    rsum = wkp.tile([P, 1], F32, tag="rsum", name="rsum")
                    nc.scalar.activation(exps[:, :wdt], ps_s[:, :wdt],
                                         ActFn.Exp, scale=float(SCALE),
                                         accum_out=rsum[:])
                    rinv = wkp.tile([P, 1], F32, tag="rinv", name="rinv")
                    nc.vector.reciprocal(rinv[:], rsum[:])
                    probB = wkp.tile([P, 3 * P], CDT, tag="probB", name="probB")
                    nc.vector.tensor_scalar_mul(probB[:, :wdt],
                                                exps[:, :wdt], rinv[:])
                    probBs.append((probB, kj_lo))
                if sc > 0:
                    emit_out_proj_sb((sc - 1) * (SC // P) + h)
                if prev is not None:
                    emit_transp_ctx(*prev)
                if sc + 1 < NSC:
                    qproj_chain(sc + 1, xs_next[0], h)
                prev = (h, pb, probBs)
            emit_transp_ctx(*prev)

        for s2 in range(SC // P):
            emit_out_proj_sb((NSC - 1) * (SC // P) + s2)

    nc.compile()
    return nc


def _band_mask():
    i = np.arange(P)[:, None]
    j = np.arange(3 * P)[None, :]
    return np.where((j >= i) & (j <= i + WIN), 0.0, -1e6).astype(np.float32)


def kernel(q, k, v, Wq, bq, Wk, bk, Wv, bv, Wo, bo):
    global LAST_RESULTS
    q = np.asarray(q, np.float32)
    k = np.asarray(k, np.float32)
    v = np.asarray(v, np.float32)
    Wq = np.asarray(Wq, np.float32)
    Wk = np.asarray(Wk, np.float32)
    Wv = np.asarray(Wv, np.float32)
    Wo = np.asarray(Wo, np.float32)
    bq = np.asarray(bq, np.float32)
    bk = np.asarray(bk, np.float32)
    bv = np.asarray(bv, np.float32)
    bo = np.asarray(bo, np.float32)

    if "nc" not in _CACHE:
        _CACHE["nc"] = _build_nc()
    nc = _CACHE["nc"]
    from concourse.bass_utils import run_bass_kernel_spmd

    if _mm_dtype_name() == "bf16":
        import ml_dtypes

        cdt = ml_dtypes.bfloat16
    else:
        cdt = np.float32

    NQ = 4
    mask = _band_mask()
    ident = np.eye(P, dtype=cdt)

    def x_block(x):
        # x[b] (S, D) -> blocked [NSC, NQ, P, 4*SC]: per-partition
        # lines are 4KB contiguous (one descriptor per partition)
        xT = x.T.astype(cdt)  # (D, S)
        return np.ascontiguousarray(
            xT.reshape(NQ, 4, P, NSC, SC).transpose(3, 0, 2, 1, 4)
            .reshape(NSC, NQ, P, 4 * SC))

    def w_block(w):
        # W slice (D, GD) -> [NQ, P, 4*GD]
        return np.ascontiguousarray(
            w.astype(cdt).reshape(NQ, 4, P, GD).transpose(0, 2, 1, 3)
            .reshape(NQ, P, 4 * GD))

    xB = {}
    for b in range(B):
        xB[("q", b)] = x_block(q[b])
        xB[("k", b)] = x_block(k[b])
        xB[("v", b)] = x_block(v[b])

    in_maps = []
    for core in range(8):
        b, g = divmod(core, 4)
        sl = slice(g * GD, (g + 1) * GD)
        in_maps.append({
            "xq_b": xB[("q", b)],
            "xk_b": xB[("k", b)],
            "xv_b": xB[("v", b)],
            "wq_b": w_block(Wq[:, sl]),
            "wk_b": w_block(Wk[:, sl]),
            "wv_b": np.ascontiguousarray(
                Wv[:, sl].astype(cdt).reshape(NDB, P, GD).transpose(1, 0, 2)
                .reshape(P, NDB * GD)),
            "wo_b": np.ascontiguousarray(
                Wo[sl, :].astype(cdt).reshape(NH, P, D).transpose(1, 0, 2)
                .reshape(P, NH * D)),
            "bq2": np.ascontiguousarray(bq[sl].reshape(NH, P).T),
            "bk2": np.ascontiguousarray(bk[sl].reshape(NH, P).T),
            "bvb": np.ascontiguousarray(np.broadcast_to(bv[sl], (P, GD))),
            "maskd": mask,
            "identd": ident,
        })

    trace = os.environ.get("KERNEL_TRACE", "0") == "1"
    res = run_bass_kernel_spmd(nc, in_maps, core_ids=list(range(8)), trace=trace)
    LAST_RESULTS = res

    out = np.zeros((B, S, D), np.float64)
    kp = np.empty((B, S, D), np.float32)
    vp = np.empty((B, S, D), np.float32)
    for core in range(8):
        b, g = divmod(core, 4)
        sl = slice(g * GD, (g + 1) * GD)
        r = res.results[core]
        kp[b][:, sl] = r["kpT"].astype(np.float32).T
        vp[b][:, sl] = r["vp"].astype(np.float32)
        out[b] += r["pout"].reshape(S, D).astype(np.float64)
    out = (out + bo.astype(np.float64)).astype(np.float32)
    return out, kp, vp


# revision 52
# speedup vs baseline: 1.1889x; 1.0014x over previous
"""Trainium2 Bass kernel for a sparse (sliding-window) attention layer.

Reference computation (B=2, S=2048, D=2048, H=16 heads, window=256, fp32):
    qp = q @ Wq + bq ; kp = k @ Wk + bk ; vp = v @ Wv + bv
    per-head scores with mask (0 <= q_idx - k_idx <= 256), softmax, ctx
    out = merge_heads(ctx) @ Wo + bo
    returns (out, kp, vp)

Sharding: 8 cores = 2 (batch) x 4 (head groups of 4 heads / 512 dims).
Each core computes its batch's projections for its 512 output dims
(transposed layout for q/k so attention feeds straight into the PE),
the windowed attention for its 4 heads, and a partial out-projection
(rows of Wo owned by its heads).  Host sums the 4 partial outputs per
batch (the "out_proj all-reduce") and concatenates kp/vp slices.

The kernel is a 4-round pipeline over 512-token seq chunks: each round
projects q/k/v for the chunk, runs the windowed attention for the
chunk's 4 query blocks on all 4 heads, and emits the chunk's partial
out-projection.  This keeps the PE array streaming continuously (no
HAM re-throttle) and spreads DVE/ACT/DMA work evenly.

Engine budget choices (PE is the bottleneck at ~90% busy, so every
non-GEMM op is pushed off the Tensor engine):
  - the sliding-window mask is ADDED ON THE VECTOR ENGINE (one
    tensor_tensor on the score PSUM against a static fp32 band-mask
    tile) instead of PE identity-matmuls
  - the prob transposes (row-major softmax probs -> k-major for the
    ctx matmul) stay on the PE (a DMA-XBAR transpose was tried and
    runs ~1.2us SERIALIZED on the issuing hwdge queue - far worse)
  - the padded prob buffers are zeroed ONCE: pad blocks are never
    written by any round
  - host-side blocked DRAM layouts give every big DMA 4-16KB
    contiguous per-partition lines (fewer descriptors, faster ramp)
  - identity/masks come from DRAM (no gpsimd make_identity on the
    startup critical path); input doorbells ride the sync queue while
    weight/output doorbells ride the scalar queue
  - kp/vp/pout DRAM outputs are bf16 (host upconverts); matmuls are
    bf16 with fp32 PSUM accumulation; softmax statistics stay fp32
"""

import os
import sys

import numpy as np

B = 2
S = 2048
D = 2048
GD = 512          # dims per core (4 heads x 128)
NH = 4            # heads per core
P = 128
WIN = 256         # sliding window
NDB = D // P      # 16 contraction blocks
SC = 512          # seq chunk (one pipeline round)
NSC = S // SC     # 4 rounds
NSB = S // P      # 16 seq blocks
SCALE = 1.0 / np.sqrt(P)

_CACHE = {}
LAST_RESULTS = None


def _mm_dtype_name():
    return os.environ.get("KERNEL_MM_DT", "bf16")


def _build_nc():
    sys.path.insert(0, "/opt/trn_rl_repo")
    import concourse.bass as bass  # noqa: F401
    import concourse.tile as tile
    from concourse import mybir, bacc
    from contextlib import ExitStack

    F32 = mybir.dt.float32
    CDT = mybir.dt.bfloat16 if _mm_dtype_name() == "bf16" else F32

    nc = bacc.Bacc("TRN2", target_bir_lowering=False, debug=False, num_devices=8)

    NQ = 4            # weight/x sub-tiles (4 db blocks each)

    # blocked DRAM layouts: 4KB+ contiguous per-partition lines
    xq_b = nc.dram_tensor("xq_b", [NSC, NQ, P, 4 * SC], CDT, kind="ExternalInput")
    xk_b = nc.dram_tensor("xk_b", [NSC, NQ, P, 4 * SC], CDT, kind="ExternalInput")
    xv_b = nc.dram_tensor("xv_b", [NSC, NQ, P, 4 * SC], CDT, kind="ExternalInput")
    wq_b = nc.dram_tensor("wq_b", [NQ, P, 4 * GD], CDT, kind="ExternalInput")
    wk_b = nc.dram_tensor("wk_b", [NQ, P, 4 * GD], CDT, kind="ExternalInput")
    wv_b = nc.dram_tensor("wv_b", [P, NDB * GD], CDT, kind="ExternalInput")
    wo_b = nc.dram_tensor("wo_b", [P, NH * D], CDT, kind="ExternalInput")
    bq2 = nc.dram_tensor("bq2", [P, NH], F32, kind="ExternalInput")
    bk2 = nc.dram_tensor("bk2", [P, NH], F32, kind="ExternalInput")
    bvb = nc.dram_tensor("bvb", [P, GD], F32, kind="ExternalInput")
    maskd = nc.dram_tensor("maskd", [P, 3 * P], F32, kind="ExternalInput")
    identd = nc.dram_tensor("identd", [P, P], CDT, kind="ExternalInput")

    kpT_o = nc.dram_tensor("kpT", [GD, S], CDT, kind="ExternalOutput")
    vp_o = nc.dram_tensor("vp", [S, GD], CDT, kind="ExternalOutput")
    pout_o = nc.dram_tensor("pout", [NSB, P, D], CDT, kind="ExternalOutput")

    kpT_r = kpT_o.ap().rearrange("(h p) s -> h p s", p=P)

    AluOp = mybir.AluOpType
    ActFn = mybir.ActivationFunctionType

    with tile.TileContext(nc) as tc, ExitStack() as top:
        const = top.enter_context(tc.tile_pool(name="const", bufs=1))
        ident = const.tile([P, P], CDT, name="ident")
        junk = const.tile([P, P], CDT, name="junk")
        nc.gpsimd.memset(junk[:], 0.125)
        nc.scalar.dma_start(ident[:], identd.ap())
        # remaining const doorbells are deferred until after the first
        # weight quarters so the scalar hwdge queue services the
        # startup-critical transfers first
        mask_sb = const.tile([P, 3 * P], F32, name="mask_sb")
        bq_sb = const.tile([P, NH], F32, name="bq_sb")
        bk_sb = const.tile([P, NH], F32, name="bk_sb")
        bvb_sb = const.tile([P, GD], F32, name="bvb_sb")

        # weights: q/k split into 4 sub-tiles so the first matmuls can
        # start as soon as the first quarter + first x quarter land.
        # Weight doorbells go on the scalar queue, x doorbells on the
        # sync queue, so both streams ramp in parallel.
        wpool = top.enter_context(tc.tile_pool(name="wpool", bufs=1))
        wq_sb = [wpool.tile([P, NDB // NQ, GD], CDT, name=f"wq_sb{i}")
                 for i in range(NQ)]
        wk_sb = [wpool.tile([P, NDB // NQ, GD], CDT, name=f"wk_sb{i}")
                 for i in range(NQ)]
        wv_sb = wpool.tile([P, NDB, GD], CDT, name="wv_sb")
        wo_sb = wpool.tile([P, NH, D], CDT, name="wo_sb")

        # long-lived activations
        persist1 = top.enter_context(tc.tile_pool(name="persist1", bufs=1))
        qpT = [persist1.tile([P, S], CDT, name=f"qpT{h}") for h in range(NH)]
        kpT = [persist1.tile([P, S], CDT, name=f"kpT{h}") for h in range(NH)]
        persist2 = top.enter_context(tc.tile_pool(name="persist2", bufs=1))
        vpB = [persist2.tile([P, GD], CDT, name=f"vpB{sb}") for sb in range(NSB)]
        persist3 = top.enter_context(tc.tile_pool(name="persist3", bufs=1))
        ctxT = [persist3.tile([P, S], CDT, name=f"ctxT{h}") for h in range(NH)]

        # transposed-prob buffers: [key-block rel 0..5, query 0..511].
        # Pad blocks (rel<t or rel>t+2) are never written by any round,
        # so a single memset keeps them zero for the whole kernel.
        persist4 = top.enter_context(tc.tile_pool(name="persist4", bufs=1))
        pbufs = [persist4.tile([P, 6, SC], CDT, name=f"pbuf{i}") for i in range(2)]
        for pb in pbufs:
            nc.gpsimd.memset(pb[:], 0.0)

        # working pools
        xpool = top.enter_context(tc.tile_pool(name="xpool", bufs=2))
        wkp = top.enter_context(tc.tile_pool(name="wkp", bufs=8))
        cpool = top.enter_context(tc.tile_pool(name="cpool", bufs=6))
        # PSUM: 4 (proj/out-proj) + 2 (scores) + 2 (transpose+ctx) = 8
        # banks.  4 proj banks let chunk 0 run four concurrent hb
        # accumulation chains paced by the arriving weight/x quarters,
        # and give the DVE bias-add/copy consumers 3 chains of slack.
        psP = top.enter_context(tc.tile_pool(name="psP", bufs=4, space="PSUM"))
        psb = top.enter_context(tc.tile_pool(name="psb", bufs=2, space="PSUM"))
        pst = top.enter_context(tc.tile_pool(name="pst", bufs=2, space="PSUM"))

        # warm the PE (HAM un-throttles after ~3.4us of activity) with
        # garbage matmuls on a memset tile: gpsimd can produce it at
        # ~6us (engine program start) while the ident DMA only lands at
        # ~10us, so warm-up begins ~4us earlier
        warm = psP.tile([P, SC], F32, tag="proj", name="warm")
        for _ in range(28):
            nc.tensor.matmul(warm[:, :P], lhsT=junk[:],
                             rhs=junk[:], start=True, stop=True)

        def emit_out_proj_sb(sb):
            # partial out-projection for one seq block (emitted one
            # round late, interleaved per attention head: its matmuls
            # are always-ready PE work that absorbs the attention
            # phase's softmax latency in the in-order PE queue)
            for ec in range(D // SC):
                psq = psP.tile([P, SC], F32, tag="proj", name="psq")
                for h in range(NH):
                    nc.tensor.matmul(
                        psq[:],
                        lhsT=ctxT[h][:, sb * P:(sb + 1) * P],
                        rhs=wo_sb[:, h, ec * SC:(ec + 1) * SC],
                        start=(h == 0),
                        stop=(h == NH - 1),
                    )
                po = cpool.tile([P, SC], CDT, tag="po", name="po")
                if ec % 2 == 0:
                    nc.scalar.copy(po[:], psq[:])
                else:
                    nc.vector.tensor_copy(po[:], psq[:])
                nc.scalar.dma_start(
                    pout_o.ap()[sb, :, ec * SC:(ec + 1) * SC], po[:])

        def load_x_chunk(x_b, sc):
            subs = []
            for i in range(NQ):
                t = xpool.tile([P, NDB // NQ, SC], CDT, tag=f"x{i}", name=f"x{i}")
                nc.sync.dma_start(t[:], x_b.ap()[sc, i])
                subs.append(t)
            return subs

        def qproj_chain(sc, xs, hb):
            # one q-projection hb chain for chunk sc (interleaved into
            # the PREVIOUS chunk's attention as always-ready PE filler)
            ps = psP.tile([P, SC], F32, tag="proj", name="ps")
            for db in range(NDB):
                nc.tensor.matmul(
                    ps[:],
                    lhsT=wq_sb[db // 4][:, db % 4, hb * P:(hb + 1) * P],
                    rhs=xs[db // 4][:, db % 4, :],
                    start=(db == 0),
                    stop=(db == NDB - 1),
                )
            nc.scalar.activation(qpT[hb][:, sc * SC:(sc + 1) * SC], ps[:],
                                 ActFn.Identity, bias=bq_sb[:, hb:hb + 1])

        xs_next = None
        for sc in range(NSC):
            ssl = slice(sc * SC, (sc + 1) * SC)
            kpT_writes = []
            if sc == 1:
                # chunk 0's output writes, deferred past the congested
                # chunk-0/prefetch DMA window (outputs have no reader)
                for hb in range(NH):
                    nc.scalar.dma_start(kpT_r[hb][:, 0:SC], kpT[hb][:, 0:SC])
                for s2 in range(SC // P):
                    nc.scalar.dma_start(
                        vp_o.ap()[s2 * P:(s2 + 1) * P, :], vpB[s2][:])

            # ---- q / k projections for this chunk (transposed layout) ----
            # (for sc>0 the q projection was already emitted, interleaved
            # into chunk sc-1's attention)
            for x_b, w_b, w_sb, b_sb, dstT, is_k in (
                (xq_b, wq_b, wq_sb, bq_sb, qpT, False),
                (xk_b, wk_b, wk_sb, bk_sb, kpT, True),
            ):
                if sc > 0 and not is_k:
                    continue
                if sc == 0:
                    # interleave weight-quarter / x-quarter transfers;
                    # the very first quarter goes db-block by db-block
                    # so the first matmul's dependencies are minimal
                    xs = []
                    for i in range(NQ):
                        t = xpool.tile([P, NDB // NQ, SC], CDT,
                                       tag=f"x{i}", name=f"x{i}")
                        if i == 0 and not is_k:
                            for d2 in range(4):
                                csl = slice(d2 * SC, (d2 + 1) * SC)
                                nc.scalar.dma_start(
                                    w_sb[0][:, d2], w_b.ap()[0][:, csl])
                                nc.sync.dma_start(
                                    t[:, d2], x_b.ap()[0, 0][:, csl])
                        else:
                            nc.scalar.dma_start(w_sb[i][:], w_b.ap()[i])
                            nc.sync.dma_start(t[:], x_b.ap()[0, i])
                        xs.append(t)
                    if not is_k:
                        # bias doorbells MUST be emitted before their
                        # first consumer (deps follow program order)
                        nc.scalar.dma_start(bq_sb[:], bq2.ap())
                        nc.scalar.dma_start(bk_sb[:], bk2.ap())
                    # quarter-major: four concurrent hb accumulation
                    # chains, paced by the arriving quarters (a single
                    # hb chain would consume input 4x faster than the
                    # DMA can deliver it)
                    ps_l = [psP.tile([P, SC], F32, tag="proj",
                                     name=f"ps{hb}") for hb in range(NH)]
                    for i in range(NQ):
                        for hb in range(NH):
                            for d2 in range(4):
                                db = 4 * i + d2
                                nc.tensor.matmul(
                                    ps_l[hb][:],
                                    lhsT=w_sb[i][:, d2, hb * P:(hb + 1) * P],
                                    rhs=xs[i][:, d2, :],
                                    start=(db == 0),
                                    stop=(db == NDB - 1),
                                )
                    for hb in range(NH):
                        nc.scalar.activation(dstT[hb][:, ssl], ps_l[hb][:],
                                             ActFn.Identity,
                                             bias=b_sb[:, hb:hb + 1])
                        if is_k:
                            kpT_writes.append(hb)
                    continue
                xs = xs_next[0 if not is_k else 1]
                for hb in range(NH):
                    ps = psP.tile([P, SC], F32, tag="proj", name="ps")
                    for db in range(NDB):
                        nc.tensor.matmul(
                            ps[:],
                            lhsT=w_sb[db // 4][:, db % 4, hb * P:(hb + 1) * P],
                            rhs=xs[db // 4][:, db % 4, :],
                            start=(db == 0),
                            stop=(db == NDB - 1),
                        )
                    nc.scalar.activation(dstT[hb][:, ssl], ps[:],
                                         ActFn.Identity,
                                         bias=b_sb[:, hb:hb + 1])
                    if is_k:
                        # doorbell deferred: a dependent output DMA at
                        # the head of the scalar queue would block the
                        # wv/wo/const doorbells behind it
                        kpT_writes.append(hb)

            # ---- v projection for this chunk (natural layout) ----
            if sc == 0:
                nc.scalar.dma_start(wv_sb[:], wv_b.ap())
                nc.scalar.dma_start(bvb_sb[:], bvb.ap())
                nc.scalar.dma_start(mask_sb[:], maskd.ap())
                xvs = load_x_chunk(xv_b, 0)
            else:
                xvs = xs_next[2]
            if sc > 0:
                for hb in kpT_writes:
                    nc.scalar.dma_start(kpT_r[hb][:, ssl], kpT[hb][:, ssl])
            if sc == 0:
                # quarter-major (DMA-paced) like the chunk-0 q/k proj
                ps2_l = [psP.tile([P, GD], F32, tag="proj",
                                  name=f"ps2_{s2}") for s2 in range(SC // P)]
                for i in range(NQ):
                    for s2 in range(SC // P):
                        for d2 in range(4):
                            db = 4 * i + d2
                            nc.tensor.matmul(
                                ps2_l[s2][:],
                                lhsT=xvs[i][:, d2, s2 * P:(s2 + 1) * P],
                                rhs=wv_sb[:, db, :],
                                start=(db == 0),
                                stop=(db == NDB - 1),
                            )
                for s2 in range(SC // P):
                    nc.vector.tensor_tensor(
                        vpB[s2][:], ps2_l[s2][:], bvb_sb[:], AluOp.add)
            else:
                for s2 in range(SC // P):
                    sb = sc * (SC // P) + s2
                    ps2 = psP.tile([P, GD], F32, tag="proj", name="ps2")
                    for db in range(NDB):
                        nc.tensor.matmul(
                            ps2[:],
                            lhsT=xvs[db // 4][:, db % 4, s2 * P:(s2 + 1) * P],
                            rhs=wv_sb[:, db, :],
                            start=(db == 0),
                            stop=(db == NDB - 1),
                        )
                    nc.vector.tensor_tensor(
                        vpB[sb][:], ps2[:], bvb_sb[:], AluOp.add)
                    nc.scalar.dma_start(
                        vp_o.ap()[sb * P:(sb + 1) * P, :], vpB[sb][:])

            # wo is first consumed by chunk 1's out-projection (~100us
            # in): fetch it during chunk 0's attention instead of
            # competing with the startup-critical q/k/v traffic.  The
            # chunk-0 kpT/vp output writes are deferred here for the
            # same reason (they are not latency-sensitive).
            if sc == 0:
                nc.scalar.dma_start(wo_sb[:], wo_b.ap())
            # prefetch next chunk's x while attention runs
            if sc + 1 < NSC:
                xs_next = (load_x_chunk(xq_b, sc + 1),
                           load_x_chunk(xk_b, sc + 1),
                           load_x_chunk(xv_b, sc + 1))

            # ---- windowed attention for this chunk's 4 query blocks ----
            qc = sc
            kjbase = 4 * qc - 2

            def emit_transp_ctx(h, pb, probBs):
                # transposes grouped per destination key-block, then
                # the ctx matmuls; called one head LATE so head h+1's
                # always-ready score matmuls sit ahead of these
                # softmax-gated instructions in the PE queue
                for rel in range(2 if qc == 0 else 0, 6):
                    t0 = max(0, rel - 2)
                    t1 = min(3, rel)
                    ps_t = pst.tile([P, 3 * P], CDT, tag="ps_t", name="ps_t")
                    for t in range(t0, t1 + 1):
                        probB, kj_lo = probBs[t]
                        j = (kjbase + rel) - kj_lo // P
                        nc.tensor.transpose(
                            ps_t[:, (t - t0) * P:(t - t0 + 1) * P],
                            probB[:, j * P:(j + 1) * P],
                            ident[:])
                    nc.vector.tensor_copy(
                        pb[:, rel, t0 * P:(t1 + 1) * P],
                        ps_t[:, 0:(t1 - t0 + 1) * P])
                ps_pv = psP.tile([P, SC], F32, tag="proj", name="ps_pv")
                for pr in range(2):
                    cs = slice(pr * 2 * P, (pr + 1) * 2 * P)
                    rels = [r for r in range(2 * pr, 2 * pr + 4)
                            if kjbase + r >= 0]
                    for i2, r in enumerate(rels):
                        nc.tensor.matmul(
                            ps_pv[:, cs],
                            lhsT=vpB[kjbase + r][:, h * P:(h + 1) * P],
                            rhs=pb[:, r, cs],
                            start=(i2 == 0),
                            stop=(i2 == len(rels) - 1),
                        )
                nc.scalar.copy(ctxT[h][:, qc * SC:(qc + 1) * SC], ps_pv[:])

            prev = None
            for h in range(NH):
                pb = pbufs[(qc * NH + h) % 2]
                # the deferred transpose+ctx group goes FIRST: its
                # PSUM->SBUF prob copies then lead the DVE queue instead
                # of sitting behind this head's masks/normalizes, so the
                # ctx matmuls (right behind the transposes in the
                # in-order PE queue) stop stalling on them
                if prev is not None:
                    emit_transp_ctx(*prev)
                probBs = []
                for t in range(4):
                    qb = qc * 4 + t
                    qs = qb * P
                    kj_lo = max(0, qs - WIN)
                    wdt = qs + P - kj_lo          # 128 / 256 / 384
                    ps_s = psb.tile([P, 3 * P], F32, tag="ps_s", name="ps_s")
                    nc.tensor.matmul(
                        ps_s[:, :wdt],
                        lhsT=qpT[h][:, qs:qs + P],
                        rhs=kpT[h][:, kj_lo:kj_lo + wdt],
                        start=True,
                        stop=True,
                    )
                    # sliding-window mask on the Vector engine (the
                    # right-aligned slice of the static band mask
                    # matches every wdt)
                    nc.vector.tensor_tensor(
                        ps_s[:, :wdt], ps_s[:, :wdt],
                        mask_sb[:, 3 * P - wdt:], AluOp.add)
                    exps = wkp.tile([P, 3 * P], CDT, tag="exps", name="exps")
                    rsum = wkp.tile([P, 1], F32, tag="rsum", name="rsum")
                    nc.scalar.activation(exps[:, :wdt], ps_s[:, :wdt],
                                         ActFn.Exp, scale=float(SCALE),
                                         accum_out=rsum[:])
                    rinv = wkp.tile([P, 1], F32, tag="rinv", name="rinv")
                    nc.vector.reciprocal(rinv[:], rsum[:])
                    probB = wkp.tile([P, 3 * P], CDT, tag="probB", name="probB")
                    nc.vector.tensor_scalar_mul(probB[:, :wdt],
                                                exps[:, :wdt], rinv[:])
                    probBs.append((probB, kj_lo))
                if sc > 0:
                    emit_out_proj_sb((sc - 1) * (SC // P) + h)
                if sc + 1 < NSC:
                    qproj_chain(sc + 1, xs_next[0], h)
                prev = (h, pb, probBs)
            emit_transp_ctx(*prev)

        for s2 in range(SC // P):
            emit_out_proj_sb((NSC - 1) * (SC // P) + s2)

    nc.compile()
    return nc


def _band_mask():
    i = np.arange(P)[:, None]
    j = np.arange(3 * P)[None, :]
    return np.where((j >= i) & (j <= i + WIN), 0.0, -1e6).astype(np.float32)


def kernel(q, k, v, Wq, bq, Wk, bk, Wv, bv, Wo, bo):
    global LAST_RESULTS
    q = np.asarray(q, np.float32)
    k = np.asarray(k, np.float32)
    v = np.asarray(v, np.float32)
    Wq = np.asarray(Wq, np.float32)
    Wk = np.asarray(Wk, np.float32)
    Wv = np.asarray(Wv, np.float32)
    Wo = np.asarray(Wo, np.float32)
    bq = np.asarray(bq, np.float32)
    bk = np.asarray(bk, np.float32)
    bv = np.asarray(bv, np.float32)
    bo = np.asarray(bo, np.float32)

    if "nc" not in _CACHE:
        _CACHE["nc"] = _build_nc()
    nc = _CACHE["nc"]
    from concourse.bass_utils import run_bass_kernel_spmd

    if _mm_dtype_name() == "bf16":
        import ml_dtypes

        cdt = ml_dtypes.bfloat16
    else:
        cdt = np.float32

    NQ = 4
    mask = _band_mask()
    ident = np.eye(P, dtype=cdt)

    def x_block(x):
        # x[b] (S, D) -> blocked [NSC, NQ, P, 4*SC]: per-partition
        # lines are 4KB contiguous (one descriptor per partition)
        xT = x.T.astype(cdt)  # (D, S)
        return np.ascontiguousarray(
            xT.reshape(NQ, 4, P, NSC, SC).transpose(3, 0, 2, 1, 4)
            .reshape(NSC, NQ, P, 4 * SC))

    def w_block(w):
        # W slice (D, GD) -> [NQ, P, 4*GD]
        return np.ascontiguousarray(
            w.astype(cdt).reshape(NQ, 4, P, GD).transpose(0, 2, 1, 3)
            .reshape(NQ, P, 4 * GD))

    xB = {}
    for b in range(B):
        xB[("q", b)] = x_block(q[b])
        xB[("k", b)] = x_block(k[b])
        xB[("v", b)] = x_block(v[b])

    in_maps = []
    for core in range(8):
        b, g = divmod(core, 4)
        sl = slice(g * GD, (g + 1) * GD)
        in_maps.append({
            "xq_b": xB[("q", b)],
            "xk_b": xB[("k", b)],
            "xv_b": xB[("v", b)],
            "wq_b": w_block(Wq[:, sl]),
            "wk_b": w_block(Wk[:, sl]),
            "wv_b": np.ascontiguousarray(
                Wv[:, sl].astype(cdt).reshape(NDB, P, GD).transpose(1, 0, 2)
                .reshape(P, NDB * GD)),
            "wo_b": np.ascontiguousarray(
                Wo[sl, :].astype(cdt).reshape(NH, P, D).transpose(1, 0, 2)
                .reshape(P, NH * D)),
            "bq2": np.ascontiguousarray(bq[sl].reshape(NH, P).T),
            "bk2": np.ascontiguousarray(bk[sl].reshape(NH, P).T),
            "bvb": np.ascontiguousarray(np.broadcast_to(bv[sl], (P, GD))),
            "maskd": mask,
            "identd": ident,
        })

    trace = os.environ.get("KERNEL_TRACE", "0") == "1"
    res = run_bass_kernel_spmd(nc, in_maps, core_ids=list(range(8)), trace=trace)
    LAST_RESULTS = res

    out = np.zeros((B, S, D), np.float64)
    kp = np.empty((B, S, D), np.float32)
    vp = np.empty((B, S, D), np.float32)
    for core in range(8):
        b, g = divmod(core, 4)
        sl = slice(g * GD, (g + 1) * GD)
        r = res.results[core]
        kp[b][:, sl] = r["kpT"].astype(np.float32).T
        vp[b][:, sl] = r["vp"].astype(np.float32)
        out[b] += r["pout"].reshape(S, D).astype(np.float64)
    out = (out + bo.astype(np.float64)).astype(np.float32)
    return out, kp, vp
